# revision 6
# baseline (speedup 1.0000x reference)
"""MultiHeadDiffAttention Trainium2 kernel (8 NeuronCores).

Sharding: batch (4) x head-group (2 groups of 8 heads) = 8 cores.
Each core computes a partial (T, C) c_proj output for its batch element
restricted to its 8 heads; the host sums the two head-group partials per
batch element.

Per-core pipeline (all matmuls on PE, fp32r for fp32 data, fp16 for the
attention probabilities / V):
  1. PE-transpose x[b] -> xT (C on partitions).
  2. Projections: Q1/Q2 and K1/K2 in (head_dim, T) layout (weights are
     host-interleaved so each 128-row chunk = one head's [q1|q2] dims);
     V in (T, vdim) layout with an appended ones column.
  3. Per head/stream: scores S^T(k,q) = K^T-tiles x Q^T (contract d=64),
     exp via ScalarE (fused 1/8 scale) -> fp16 P, causal mask via
     gpsimd memset/affine_select, then PV: P-tile^T x [V|1] accumulated
     over k-tiles gives Y and the softmax denominator in one matmul.
  4. Streams combined as z = Y1 - (lam*den1/den2) * Y2 (per-q scalars),
     which equals den1 * (a1 - lam*a2) @ V; LayerNorm is scale-invariant
     per row, so normalizing z with eps scaled by den1^2 reproduces the
     reference exactly.
  5. LN via bn_stats/bn_aggr + exp(-0.5*ln(var+eps*den1^2) + ln(1-li)).
  6. PE-transpose y_ln, c_proj vs host-sliced Wc rows -> partial out.
"""

import contextlib
import ctypes
import math
import sys
import types

import numpy as np

sys.path.insert(0, "/opt/trn_rl_repo")


def _install_ntff_hook():
    """Provide antenv.axon_hooks if the image lacks it (for trace=True)."""
    try:
        from antenv.axon_hooks import get_axon_ntff_profile_hook  # noqa: F401

        return
    except ImportError:
        pass

    so_path = "/opt/axon/libaxon_pjrt.so"

    def _make_hook():
        try:
            lib = ctypes.CDLL(so_path)
        except OSError:
            return None
        if not hasattr(lib, "axon_start_nrt_profile"):
            return None
        lib.axon_start_nrt_profile.argtypes = [
            ctypes.POINTER(ctypes.c_int64),
            ctypes.c_size_t,
        ]
        lib.axon_start_nrt_profile.restype = ctypes.c_int64
        lib.axon_stop_nrt_profile.argtypes = [ctypes.c_char_p]
        lib.axon_stop_nrt_profile.restype = ctypes.c_int64

        @contextlib.contextmanager
        def _hook(output_dir, device_ids):
            import jax

            jax.devices()
            if device_ids:
                ids = (ctypes.c_int64 * len(device_ids))(*device_ids)
                rc = lib.axon_start_nrt_profile(ids, len(device_ids))
            else:
                rc = lib.axon_start_nrt_profile(None, 0)
            if rc != 0:
                raise RuntimeError(f"axon_start_nrt_profile rc={rc}")
            try:
                yield
            finally:
                n = lib.axon_stop_nrt_profile(str(output_dir).encode())
                if n < 0:
                    raise RuntimeError(f"axon_stop_nrt_profile rc={n}")

        return _hook

    mod = types.ModuleType("antenv.axon_hooks")
    _the_hook = _make_hook()
    mod.get_axon_ntff_profile_hook = lambda: _the_hook
    sys.modules["antenv.axon_hooks"] = mod


_install_ntff_hook()

import concourse.bass as bass  # noqa: E402
import concourse.mybir as mybir  # noqa: E402
import concourse.tile as tile  # noqa: E402
from concourse.masks import make_identity  # noqa: E402

P = 128
T = 1024
C = 1024
NH = 8  # heads per core
HS = 64
LAMBDA_INIT = 0.8 - 0.6 * math.exp(-0.3 * (2 - 1))
LN_EPS = 1e-5
N_CORES = 8

f32 = mybir.dt.float32
f32r = mybir.dt.float32r
f16 = mybir.dt.float16
Alu = mybir.AluOpType
Act = mybir.ActivationFunctionType


def r(ap):
    return ap.bitcast(f32r)


def build_program():
    nc = bass.Bass()
    x_d = nc.dram_tensor("x", [T, C], f32, kind="ExternalInput")
    wq_d = nc.dram_tensor("wq", [C, C], f32r, kind="ExternalInput")
    wk_d = nc.dram_tensor("wk", [C, C], f32r, kind="ExternalInput")
    wv_d = nc.dram_tensor("wv", [C, C], f32r, kind="ExternalInput")
    wc_d = nc.dram_tensor("wc", [C, C], f32r, kind="ExternalInput")
    lamneg_d = nc.dram_tensor("lamneg", [P, NH], f32, kind="ExternalInput")
    out_d = nc.dram_tensor("out", [T, C], f32, kind="ExternalOutput")

    ln_bias = float(math.log(1.0 - LAMBDA_INIT))

    with tile.TileContext(nc) as tc:
        with (
            tc.tile_pool(name="const", bufs=1) as const,
            tc.tile_pool(name="ydata", bufs=8) as y_pool,
        ):
            ident = const.tile([P, P], f32, tag="ident")
            make_identity(nc, ident)
            lamneg = const.tile([P, NH], f32, tag="lamneg")
            nc.sync.dma_start(out=lamneg, in_=lamneg_d[:, :])
            den_store = const.tile([P, NH, 8], f32, tag="den")
            lnb = const.tile([P, 1], f32, tag="lnb")
            nc.vector.memset(lnb, ln_bias)

            y_tiles = [y_pool.tile([P, NH * P], f32, tag="y", name="yt") for _ in range(8)]

            with tc.tile_pool(name="qkv", bufs=8) as qkv:
                q12T = [qkv.tile([P, T], f32r, tag="q", name="q12T") for _ in range(NH)]
                k12T = [qkv.tile([P, T], f32r, tag="k", name="k12T") for _ in range(NH)]
                v_aug = [qkv.tile([P, NH, 132], f16, tag="v", name="vaug") for _ in range(8)]

                # ---------- Phase A+B: x transpose + projections ----------
                with (
                    tc.tile_pool(name="xnat", bufs=2) as xnat_p,
                    tc.tile_pool(name="xT", bufs=8) as xT_p,
                    tc.tile_pool(name="wstream", bufs=8) as w_p,
                    tc.tile_pool(name="psA", bufs=2, space="PSUM") as psA,
                    tc.tile_pool(name="psB", bufs=4, space="PSUM") as psB,
                ):
                    xT = [xT_p.tile([P, T], f32r, tag="xT", name="xT") for _ in range(8)]
                    for i in range(8):
                        xn = xnat_p.tile([P, C], f32, tag="xn")
                        nc.sync.dma_start(
                            out=xn, in_=x_d[128 * i : 128 * (i + 1), :]
                        )
                        for jh in range(2):
                            pt = psA.tile([P, 512], f32, tag="psA")
                            for w in range(4):
                                j = 4 * jh + w
                                nc.tensor.transpose(
                                    out=pt[:, 128 * w : 128 * (w + 1)],
                                    in_=xn[:, 128 * j : 128 * (j + 1)],
                                    identity=ident,
                                )
                            for w in range(4):
                                j = 4 * jh + w
                                nc.any.tensor_copy(
                                    out=xT[j][:, 128 * i : 128 * (i + 1)],
                                    in_=pt[:, 128 * w : 128 * (w + 1)],
                                )

                    # V projection: out (T, vd); lhsT = xT tile, rhs = wv
                    wv_sb = [w_p.tile([P, C], f32r, tag="w", name="wsb") for _ in range(8)]
                    for c in range(8):
                        nc.sync.dma_start(
                            out=wv_sb[c], in_=wv_d[128 * c : 128 * (c + 1), :]
                        )
                    for t in range(8):
                        for n in range(2):
                            ps = psB.tile([P, 512], f32, tag="psB")
                            for c in range(8):
                                nc.tensor.matmul(
                                    ps,
                                    lhsT=(xT[c][:, 128 * t : 128 * (t + 1)]),
                                    rhs=(wv_sb[c][:, 512 * n : 512 * (n + 1)]),
                                    start=(c == 0),
                                    stop=(c == 7),
                                )
                            nc.any.tensor_copy(
                                out=v_aug[t][:, 4 * n : 4 * (n + 1), 0:128],
                                in_=ps.rearrange("p (g d) -> p g d", g=4),
                            )
                        nc.vector.memset(v_aug[t][:, :, 128:129], 1.0)

                    # Q then K projections: out (hd, T); lhsT = w tile, rhs = xT
                    for w_d, dest in ((wq_d, q12T), (wk_d, k12T)):
                        w_sb = [w_p.tile([P, C], f32r, tag="w", name="wsb") for _ in range(8)]
                        for c in range(8):
                            nc.sync.dma_start(
                                out=w_sb[c], in_=w_d[128 * c : 128 * (c + 1), :]
                            )
                        for h in range(NH):
                            for n in range(2):
                                ps = psB.tile([P, 512], f32, tag="psB")
                                for c in range(8):
                                    nc.tensor.matmul(
                                        ps,
                                        lhsT=(w_sb[c][:, 128 * h : 128 * (h + 1)]),
                                        rhs=(xT[c][:, 512 * n : 512 * (n + 1)]),
                                        start=(c == 0),
                                        stop=(c == 7),
                                    )
                                nc.any.tensor_copy(
                                    out=dest[h][:, 512 * n : 512 * (n + 1)], in_=ps
                                )

                # ---------- Phase C: attention ----------
                with (
                    tc.tile_pool(name="pprob", bufs=6) as p_pool,
                    tc.tile_pool(name="smallc", bufs=16) as small,
                    tc.tile_pool(name="psS", bufs=2, space="PSUM") as psS,
                    tc.tile_pool(name="psY", bufs=3, space="PSUM") as psY,
                ):
                    for h in range(NH):
                        pcs = {}
                        # scores + exp + mask, both streams
                        for s in range(2):
                            for n in range(2):
                                pch = p_pool.tile([P, 8, 512], f16, tag="p")
                                pcs[(s, n)] = pch
                                nk = 4 * n + 4
                                for jp in range(nk // 2):
                                    sp = psS.tile([P, 2, 512], f32, tag="psS")
                                    for u in range(2):
                                        j = 2 * jp + u
                                        nc.tensor.matmul(
                                            sp[:, u, :],
                                            lhsT=(
                                                k12T[h][
                                                    64 * s : 64 * (s + 1),
                                                    128 * j : 128 * (j + 1),
                                                ]
                                            ),
                                            rhs=(
                                                q12T[h][
                                                    64 * s : 64 * (s + 1),
                                                    512 * n : 512 * (n + 1),
                                                ]
                                            ),
                                            start=True,
                                            stop=True,
                                        )
                                    nc.scalar.activation(
                                        out=pch[:, 2 * jp : 2 * jp + 2, :],
                                        in_=sp,
                                        func=Act.Exp,
                                        scale=0.125,
                                    )
                                # causal mask on the 4 diagonal k-tiles
                                for t in range(4):
                                    j = 4 * n + t
                                    for sp_ in range(t):
                                        nc.gpsimd.memset(
                                            pch[:, j, 128 * sp_ : 128 * (sp_ + 1)],
                                            0.0,
                                        )
                                    nc.gpsimd.affine_select(
                                        out=pch[:, j, 128 * t : 128 * (t + 1)],
                                        in_=pch[:, j, 128 * t : 128 * (t + 1)],
                                        compare_op=Alu.is_ge,
                                        fill=0.0,
                                        base=0,
                                        pattern=[[1, 128]],
                                        channel_multiplier=-1,
                                    )
                        # PV + combine, stream 0 then 1
                        for s in range(2):
                            for i in range(8):
                                n, t = i // 4, i % 4
                                pch = pcs[(s, n)]
                                yp = psY.tile([P, 129], f32, tag="psY")
                                for j in range(i + 1):
                                    nc.tensor.matmul(
                                        yp,
                                        lhsT=pch[:, j, 128 * t : 128 * (t + 1)],
                                        rhs=v_aug[j][:, h, 0:129],
                                        start=(j == 0),
                                        stop=(j == i),
                                    )
                                if s == 0:
                                    nc.scalar.copy(
                                        out=y_tiles[i][:, 128 * h : 128 * (h + 1)],
                                        in_=yp[:, 0:128],
                                    )
                                    nc.vector.tensor_copy(
                                        out=den_store[:, h, i : i + 1],
                                        in_=yp[:, 128:129],
                                    )
                                else:
                                    r2 = small.tile([P, 1], f32, tag="r2")
                                    nc.vector.reciprocal(
                                        out=r2, in_=yp[:, 128:129]
                                    )
                                    gneg = small.tile([P, 1], f32, tag="gneg")
                                    nc.vector.tensor_mul(
                                        out=gneg,
                                        in0=den_store[:, h, i : i + 1],
                                        in1=r2,
                                    )
                                    nc.vector.tensor_mul(
                                        out=gneg,
                                        in0=gneg,
                                        in1=lamneg[:, h : h + 1],
                                    )
                                    tmp = small.tile([P, P], f32, tag="tmp")
                                    nc.scalar.activation(
                                        out=tmp,
                                        in_=yp[:, 0:128],
                                        func=Act.Copy,
                                        scale=gneg,
                                    )
                                    nc.vector.tensor_add(
                                        out=y_tiles[i][:, 128 * h : 128 * (h + 1)],
                                        in0=y_tiles[i][:, 128 * h : 128 * (h + 1)],
                                        in1=tmp,
                                    )

            # ---------- Phase D/E/F: LN, transpose, c_proj ----------
            with (
                tc.tile_pool(name="smalld", bufs=10) as sd,
                tc.tile_pool(name="ylnT", bufs=8) as ylnT_p,
                tc.tile_pool(name="wcp", bufs=8) as wc_p,
                tc.tile_pool(name="outp", bufs=3) as out_p,
                tc.tile_pool(name="psE", bufs=2, space="PSUM") as psE,
                tc.tile_pool(name="psF", bufs=4, space="PSUM") as psF,
            ):
                wc_sb = [wc_p.tile([P, C], f32r, tag="wc", name="wcsb") for _ in range(8)]
                for d in range(8):
                    nc.sync.dma_start(
                        out=wc_sb[d], in_=wc_d[128 * d : 128 * (d + 1), :]
                    )

                # LN stats (batch Log/Exp to avoid ACT table thrash)
                veps_tiles = []
                mu_tiles = []
                for i in range(8):
                    mu_all = sd.tile([P, NH], f32, tag="mu")
                    var_all = sd.tile([P, NH], f32, tag="var")
                    for h in range(NH):
                        bs = sd.tile([P, nc.vector.BN_STATS_DIM], f32, tag="bs")
                        nc.vector.bn_stats(
                            out=bs, in_=y_tiles[i][:, 128 * h : 128 * (h + 1)]
                        )
                        mv = sd.tile([P, nc.vector.BN_AGGR_DIM], f32, tag="mv")
                        nc.vector.bn_aggr(out=mv, in_=bs)
                        nc.vector.tensor_copy(
                            out=mu_all[:, h : h + 1], in_=mv[:, 0:1]
                        )
                        nc.vector.tensor_copy(
                            out=var_all[:, h : h + 1], in_=mv[:, 1:2]
                        )
                    d1 = den_store[:, :, i : i + 1].rearrange("p h one -> p (h one)")
                    veps = sd.tile([P, NH], f32, tag="veps")
                    nc.vector.tensor_mul(out=veps, in0=d1, in1=d1)
                    nc.vector.tensor_scalar(
                        out=veps,
                        in0=veps,
                        scalar1=LN_EPS,
                        scalar2=None,
                        op0=Alu.mult,
                    )
                    nc.vector.tensor_add(out=veps, in0=veps, in1=var_all)
                    veps_tiles.append(veps)
                    mu_tiles.append(mu_all)
                invstd_tiles = []
                for i in range(8):
                    lnv = sd.tile([P, NH], f32, tag="lnv")
                    nc.scalar.activation(
                        out=lnv, in_=veps_tiles[i], func=Act.Ln
                    )
                    invstd_tiles.append(lnv)
                for i in range(8):
                    nc.scalar.activation(
                        out=invstd_tiles[i],
                        in_=invstd_tiles[i],
                        func=Act.Exp,
                        scale=-0.5,
                        bias=lnb,
                    )
                for i in range(8):
                    for h in range(NH):
                        nc.vector.tensor_scalar(
                            out=y_tiles[i][:, 128 * h : 128 * (h + 1)],
                            in0=y_tiles[i][:, 128 * h : 128 * (h + 1)],
                            scalar1=mu_tiles[i][:, h : h + 1],
                            scalar2=invstd_tiles[i][:, h : h + 1],
                            op0=Alu.subtract,
                            op1=Alu.mult,
                        )

                # transpose y -> ylnT (d2 on partitions)
                ylnT = [ylnT_p.tile([P, T], f32r, tag="ylnT", name="ylnT") for _ in range(8)]
                for d in range(8):
                    for u in range(2):
                        pt = psE.tile([P, 512], f32, tag="psE")
                        for w in range(4):
                            i = 4 * u + w
                            nc.tensor.transpose(
                                out=pt[:, 128 * w : 128 * (w + 1)],
                                in_=y_tiles[i][:, 128 * d : 128 * (d + 1)],
                                identity=ident,
                            )
                        nc.any.tensor_copy(
                            out=ylnT[d][:, 512 * u : 512 * (u + 1)], in_=pt
                        )

                # c_proj
                for m in range(8):
                    osb = out_p.tile([P, C], f32, tag="osb")
                    for n in range(2):
                        ps = psF.tile([P, 512], f32, tag="psF")
                        for d in range(8):
                            nc.tensor.matmul(
                                ps,
                                lhsT=(ylnT[d][:, 128 * m : 128 * (m + 1)]),
                                rhs=(wc_sb[d][:, 512 * n : 512 * (n + 1)]),
                                start=(d == 0),
                                stop=(d == 7),
                            )
                        nc.any.tensor_copy(
                            out=osb[:, 512 * n : 512 * (n + 1)], in_=ps
                        )
                    nc.sync.dma_start(
                        out=out_d[128 * m : 128 * (m + 1), :], in_=osb
                    )

    bass._bass_rust.generate_event_semaphores(nc)
    return nc


_NC = None


def _get_program():
    global _NC
    if _NC is None:
        _NC = build_program()
    return _NC


def make_in_maps(inputs):
    """Host-side sharding: per-core input dicts."""
    x = np.ascontiguousarray(np.asarray(inputs["x"], dtype=np.float32))
    Wq1 = np.asarray(inputs["Wq1"], dtype=np.float32)
    Wq2 = np.asarray(inputs["Wq2"], dtype=np.float32)
    Wk1 = np.asarray(inputs["Wk1"], dtype=np.float32)
    Wk2 = np.asarray(inputs["Wk2"], dtype=np.float32)
    Wv = np.asarray(inputs["Wv"], dtype=np.float32)
    Wc = np.asarray(inputs["Wc"], dtype=np.float32)
    lq1 = np.asarray(inputs["lq1"], dtype=np.float32)
    lk1 = np.asarray(inputs["lk1"], dtype=np.float32)
    lq2 = np.asarray(inputs["lq2"], dtype=np.float32)
    lk2 = np.asarray(inputs["lk2"], dtype=np.float32)

    lam1 = np.exp(np.sum(lq1 * lk1, axis=-1))
    lam2 = np.exp(np.sum(lq2 * lk2, axis=-1))
    lam_full = (lam1 - lam2 + LAMBDA_INIT).astype(np.float32)  # (16,)

    in_maps = []
    for core in range(N_CORES):
        b, hg = core // 2, core % 2
        heads = np.arange(NH) + NH * hg  # global head idx
        wq = np.empty((C, C), np.float32)
        wk = np.empty((C, C), np.float32)
        wv = np.empty((C, C), np.float32)
        for h in range(NH):
            H = NH * hg + h
            wq[:, 128 * h : 128 * h + 64] = Wq1[:, HS * H : HS * (H + 1)]
            wq[:, 128 * h + 64 : 128 * (h + 1)] = Wq2[:, HS * H : HS * (H + 1)]
            wk[:, 128 * h : 128 * h + 64] = Wk1[:, HS * H : HS * (H + 1)]
            wk[:, 128 * h + 64 : 128 * (h + 1)] = Wk2[:, HS * H : HS * (H + 1)]
            wv[:, 128 * h : 128 * (h + 1)] = Wv[:, 128 * H : 128 * (H + 1)]
        wc = np.ascontiguousarray(Wc[1024 * hg : 1024 * (hg + 1), :])
        lamneg = np.broadcast_to(
            -lam_full[heads][None, :], (P, NH)
        ).astype(np.float32)
        in_maps.append(
            {
                "x": np.ascontiguousarray(x[b]),
                "wq": wq,
                "wk": wk,
                "wv": wv,
                "wc": wc,
                "lamneg": np.ascontiguousarray(lamneg),
            }
        )
    return in_maps


def run(inputs, trace=False, **kw):
    from concourse.bass_utils import run_bass_kernel_spmd

    nc = _get_program()
    in_maps = make_in_maps(inputs)
    res = run_bass_kernel_spmd(
        nc, in_maps, core_ids=list(range(N_CORES)), trace=trace, **kw
    )
    B = 4
    out = np.empty((B, T, C), np.float32)
    for b in range(B):
        out[b] = res.results[2 * b]["out"] + res.results[2 * b + 1]["out"]
    return out, res


def kernel(**inputs) -> np.ndarray:
    out, _ = run(inputs, trace=False)
    return out
